# revision 41
# baseline (speedup 1.0000x reference)
"""Trainium2 Bass kernel for a 3-layer GCN (directional, symmetric-norm,
self-loops, skip connections, LayerNorm between layers).

Strategy (8 NeuronCores, SPMD, single NEFF launch):
  - Nodes are sharded by destination across the 8 cores (12500 each).
  - Per layer, each core computes its shard of h' = x @ W, writes it to HBM
    partition-major (so stores are flat 2D DMAs), and a per-quarter
    AllGather replicates the full h' table to every core.
  - Edges are bucketed on the host by (source-chunk, dest-window): the
    source space splits into 4 chunks so gather indices fit in int16, and
    destinations into 128-node windows.
  - Messages are fetched with the GPSIMD dma_gather custom instruction
    (256B bf16 rows); the gather calls round-robin over 4 SWDGE queues so
    descriptor generation runs on all four Q7 core pairs concurrently
    (queue q runs on Q7 cores {2q, 2q+1} - the single-queue default
    serializes at ~8 ns/edge and dominates the whole kernel).
  - Aggregation per 128-edge block is a weighted one-hot matmul on the
    TensorEngine: S[e, d] = norm(e) for d == dest_rel(e). S tiles are
    built ON THE HOST (carrying the dinv[row]*dinv[col] edge weights) and
    streamed from HBM via the Scalar-engine HWDGE, which removes all
    DVE is_equal/scale work from the inner loop. PSUM accumulates per
    (chunk, window) group into an SBUF accumulator.
  - The gather index table is layer-invariant and loaded to SBUF once.
  - Self-loops use the local bf16 h' copy scaled by dinv^2; the epilogue
    (skip add, ReLU, LayerNorm) runs node-major, stats batched per layer.

The per-(chunk,window) block counts are made identical across cores by
padding to the max (pad slots gather row 0 with zero S weight), so a single
program runs on all 8 cores with per-core data only.
"""

import math
import os
import sys

import numpy as np

for _p in ("/opt/trn_rl_repo",):
    if os.path.isdir(_p) and _p not in sys.path:
        sys.path.insert(0, _p)

import concourse.bacc as bacc
import concourse.bass as bass
import concourse.mybir as mybir
import concourse.tile as tile
from concourse.bass_utils import run_bass_kernel_spmd

try:
    from ml_dtypes import bfloat16 as np_bf16
except ImportError:  # pragma: no cover
    np_bf16 = mybir.dt.np(mybir.dt.bfloat16)

F32 = mybir.dt.float32
BF16 = mybir.dt.bfloat16
I16 = mybir.dt.int16
AOP = mybir.AluOpType


class Cfg:
    def __init__(self, N=100000, E=1600000, D=64, L=3, n_cores=8,
                 seg_edges=8192, sblk=8, msg_bf16=True, eps=1e-5,
                 dma_scratch=16384, force_single_packet=False, n_queues=4):
        self.dma_scratch = dma_scratch
        self.force_single_packet = force_single_packet
        self.n_queues = n_queues
        assert N % n_cores == 0
        self.N, self.E, self.D, self.L, self.n_cores = N, E, D, L, n_cores
        self.eps = eps
        self.npc = N // n_cores                     # nodes per core
        nt0 = (self.npc + 127) // 128               # 128-node tiles (= windows)
        nch = max(1, math.ceil(nt0 * 128 * n_cores / 65536))
        self.nt = ((nt0 + nch - 1) // nch) * nch    # quarter-aligned tiles
        self.t_pad = self.nt * 128                  # padded shard size
        self.n_chunks = nch
        self.qsz = self.t_pad // nch                # rows per source-quarter
        self.qsz_ag = self.qsz
        self.chunk = self.qsz_ag * n_cores          # rows per gather table
        assert self.chunk // 2 <= 32768 and self.qsz % 128 == 0
        self.seg_edges = seg_edges                  # edges per dma_gather call
        assert seg_edges % 128 == 0
        self.segblk = seg_edges // 128
        self.sblk = sblk                            # S-tiles built per DVE op
        self.msg_bf16 = msg_bf16
        self.msg_dt = BF16 if msg_bf16 else F32
        self.row_elems = 128 if msg_bf16 else 64    # gather elem_size (=256B)


def _prep(cfg, x, edge_index, Ws, bs, ln_g, ln_b):
    """Host-side preprocessing: degrees, edge bucketing, per-core arrays."""
    c = cfg
    row = np.asarray(edge_index[0], dtype=np.int64)
    col = np.asarray(edge_index[1], dtype=np.int64)
    deg = np.bincount(row, minlength=c.N).astype(np.float64) + 1.0
    dinv = (1.0 / np.sqrt(deg)).astype(np.float32)

    norm = (dinv[row] * dinv[col]).astype(np.float32)  # per-edge weight

    core = row // c.npc
    row_local = row - core * c.npc
    win = row_local >> 7
    row_rel = row_local & 127
    # source position in the partition-major gather table: node (tile t,
    # partition p) lands at row p*qt + (t % qt) of its quarter's shard
    qt_t = c.nt // c.n_chunks                         # tiles per quarter
    qt_ag = c.qsz_ag // 128                           # tiles incl AG pad
    src_core = col // c.npc
    src_loc = col % c.npc
    s_t = src_loc // 128
    s_p = src_loc % 128
    chunk = s_t // qt_t                               # source quarter
    pos = src_core * c.qsz_ag + s_p * qt_ag + (s_t % qt_t)
    idx_rel = pos >> 1                                # 256B row = node pair
    parity = (pos & 1).astype(np.int64)

    # bucket key and stable sort
    key = (core * c.n_chunks + chunk) * c.nt + win
    order = np.argsort(key, kind="stable")
    key_s = key[order]
    idx_s = idx_rel[order].astype(np.int32)
    rr_s = row_rel[order].astype(np.int32)
    nm_s = norm[order]
    par_s = parity[order]

    nbuck = c.n_cores * c.n_chunks * c.nt
    counts = np.bincount(key_s, minlength=nbuck).reshape(c.n_cores, c.n_chunks, c.nt)
    starts_flat = np.zeros(nbuck + 1, dtype=np.int64)
    np.cumsum(counts.reshape(-1), out=starts_flat[1:])

    # common (max over cores) block counts per (chunk, window)
    blocks = np.ceil(counts.max(axis=0) / 128.0).astype(np.int64)  # [n_chunks, nt]
    nblk_total = int(blocks.sum())
    e_pad = nblk_total * 128
    blk_off = np.zeros(c.n_chunks * c.nt + 1, dtype=np.int64)
    np.cumsum(blocks.reshape(-1), out=blk_off[1:])

    # per-core padded edge streams (pad edges: idx 0; S weight 0) and
    # host-built weighted one-hot S tiles: S[e%128, blk, dest] = norm(e)
    msg_np = np_bf16 if c.msg_bf16 else np.float32
    idx_arrs, s_arrs = [], []
    for cc in range(c.n_cores):
        idx_a = np.zeros(e_pad, dtype=np.int16)
        rr_a = np.zeros(e_pad, dtype=np.int64)
        nm_a = np.zeros(e_pad, dtype=np.float32)
        pa_a = np.zeros(e_pad, dtype=np.int64)
        for ck in range(c.n_chunks):
            for w in range(c.nt):
                b = starts_flat[(cc * c.n_chunks + ck) * c.nt + w]
                e = starts_flat[(cc * c.n_chunks + ck) * c.nt + w + 1]
                n = e - b
                o = blk_off[ck * c.nt + w] * 128
                idx_a[o:o + n] = idx_s[b:e].astype(np.int16)
                rr_a[o:o + n] = rr_s[b:e]
                nm_a[o:o + n] = nm_s[b:e]
                pa_a[o:o + n] = par_s[b:e]
        idx_arrs.append(idx_a)
        # parity-split weighted one-hot: S[par][e%128, blk, dest] = norm(e)
        S = np.zeros((128, nblk_total, 2, 128), dtype=msg_np)
        slot = np.arange(e_pad, dtype=np.int64)
        S[slot % 128, slot // 128, pa_a, rr_a] = nm_a.astype(msg_np)
        s_arrs.append(S)

    # wrap indices for dma_gather: flat i -> [i%16, i//16], replicated to 128 partitions
    def wrap_idx(a):
        return np.tile(a.reshape(-1, 16).T, (8, 1)).astype(np.int16)

    per_core = []
    x = np.asarray(x, dtype=np.float32)
    for cc in range(c.n_cores):
        xs = np.zeros((c.t_pad, c.D), dtype=np.float32)
        xs[:c.npc] = x[cc * c.npc:(cc + 1) * c.npc]
        # partition-major x: [128, nt*64], row p holds tiles' node t*128+p
        xpm = np.ascontiguousarray(
            xs.reshape(c.nt, 128, c.D).transpose(1, 0, 2)).reshape(128, c.nt * c.D)
        dl = np.zeros(c.t_pad, dtype=np.float32)
        dl[:c.npc] = dinv[cc * c.npc:(cc + 1) * c.npc]
        d2 = (dl * dl).astype(np.float32)
        per_core.append({
            "x_in": xpm,
            "dinv2_in": d2.reshape(c.nt, 128).T.copy(),     # [128, nt]
            "idxs_in": wrap_idx(idx_arrs[cc]),              # [128, e_pad//16]
            "S_in": s_arrs[cc].reshape(128, nblk_total * 256),  # [128, nblk*2*128]
        })

    consts = {
        "W_in": np.ascontiguousarray(np.asarray(Ws, dtype=np.float32)),   # [L,64,64]
        "i64_in": np.eye(64, dtype=np.float32),
        "i128_in": np.eye(128, dtype=np.float32),
    }
    bs = np.asarray(bs, dtype=np.float32)
    ln_g = np.asarray(ln_g, dtype=np.float32)
    ln_b = np.asarray(ln_b, dtype=np.float32)
    flags = {
        "bias": bool(np.any(bs != 0.0)),
        "affine": bool(np.any(ln_g != 1.0) or np.any(ln_b != 0.0)),
    }
    if flags["bias"]:
        consts["bs_in"] = np.tile(bs[:, None, :], (1, 128, 1))        # [L,128,64]
    if flags["affine"]:
        consts["lng_in"] = np.tile(ln_g[:, None, :], (1, 128, 1))     # [L-1,128,64]
        consts["lnb_in"] = np.tile(ln_b[:, None, :], (1, 128, 1))
    struct = {
        "blocks": blocks,            # [n_chunks, nt]
        "nblk_total": nblk_total,
        "e_pad": e_pad,
    }
    return per_core, consts, struct, flags


def _build(cfg, struct, flags):
    """Build the Bass/Tile program. Returns nc."""
    c = cfg
    blocks = struct["blocks"]
    nblk_total = struct["nblk_total"]
    e_pad = struct["e_pad"]
    D = c.D
    NT = c.nt
    MSG = c.msg_dt
    ROWE = c.row_elems

    # first chunk contributing each window (for copy-vs-add into agg)
    first_ck = [None] * NT
    for w in range(NT):
        for ck in range(c.n_chunks):
            if blocks[ck, w] > 0:
                first_ck[w] = ck
                break

    nc = bacc.Bacc("TRN2", num_devices=c.n_cores, target_bir_lowering=False,
                   debug=False, enable_asserts=False,
                   dynamic_dma_scratch_size=c.dma_scratch,
                   num_swdge_queues=c.n_queues)

    # I/O
    x_in = nc.dram_tensor("x_in", [128, (c.t_pad // 128) * D], F32, kind="ExternalInput")
    dinv2_in = nc.dram_tensor("dinv2_in", [128, NT], F32, kind="ExternalInput")
    idxs_in = nc.dram_tensor("idxs_in", [128, e_pad // 16], I16, kind="ExternalInput")
    S_in = nc.dram_tensor("S_in", [128, nblk_total * 256], MSG, kind="ExternalInput")
    W_in = nc.dram_tensor("W_in", [c.L, D, D], F32, kind="ExternalInput")
    i64_in = nc.dram_tensor("i64_in", [64, 64], F32, kind="ExternalInput")
    i128_in = nc.dram_tensor("i128_in", [128, 128], F32, kind="ExternalInput")
    if flags["bias"]:
        bs_in = nc.dram_tensor("bs_in", [c.L, 128, D], F32, kind="ExternalInput")
    if flags["affine"]:
        lng_in = nc.dram_tensor("lng_in", [c.L - 1, 128, D], F32, kind="ExternalInput")
        lnb_in = nc.dram_tensor("lnb_in", [c.L - 1, 128, D], F32, kind="ExternalInput")
    emb_out = nc.dram_tensor("emb_out", [128, (c.t_pad // 128) * D], F32, kind="ExternalOutput")
    x_out = nc.dram_tensor("x_out", [128, (c.t_pad // 128) * D], F32, kind="ExternalOutput")

    with tile.TileContext(nc) as tc:
        with (
            tc.tile_pool(name="dram", bufs=1, space="DRAM") as dram_pool,
            tc.tile_pool(name="const", bufs=1) as const_pool,
            tc.tile_pool(name="state", bufs=1) as state_pool,
            tc.tile_pool(name="xd", bufs=4) as xd_pool,
            tc.tile_pool(name="xT", bufs=2) as xT_pool,
            tc.tile_pool(name="msg", bufs=7) as msg_pool,
            tc.tile_pool(name="sS", bufs=4) as s_pool,
            tc.tile_pool(name="stats", bufs=2) as stats_pool,
            tc.tile_pool(name="pagg", bufs=5, space="PSUM") as pagg_pool,
            tc.tile_pool(name="pbig", bufs=1, space="PSUM") as pbig_pool,
            tc.tile_pool(name="ptr", bufs=2, space="PSUM") as ptr_pool,
        ):
            # ---- DRAM internal buffers for the halo exchange / gather ----
            # per-(layer, quarter) shard, partition-major: linear order is
            # (p, tile-in-quarter, col) so the AllGather rank-concat yields a
            # node-row-major gather table under the renumbered node ids.
            qt = NT // c.n_chunks
            qt_ag = c.qsz_ag // 128    # AG tiles per quarter (incl pad tiles)
            hp_shards = [
                [dram_pool.tile([128, qt_ag * 64], MSG, name=f"hp_shard{i}q{q}")
                 for q in range(c.n_chunks)]
                for i in range(c.L)]
            hp_fulls = [
                [dram_pool.tile([c.chunk // 2, ROWE], MSG, addr_space="Shared",
                                name=f"hp_full{i}q{q}")
                 for q in range(c.n_chunks)]
                for i in range(c.L)]

            # ---- constants ----
            dinv2T = const_pool.tile([128, NT], F32)
            nc.sync.dma_start(dinv2T[:], dinv2_in[:])
            idx_all = const_pool.tile([128, e_pad // 16], I16)
            nc.sync.dma_start(idx_all[:], idxs_in[:])
            i64 = const_pool.tile([64, 64], F32)
            nc.sync.dma_start(i64[:], i64_in[:])
            i128 = const_pool.tile([128, 128], F32)
            nc.sync.dma_start(i128[:], i128_in[:])
            eps_sb = const_pool.tile([128, 1], F32)
            nc.vector.memset(eps_sb[:], float(c.eps))
            W_sb = const_pool.tile([64, c.L, D], F32)
            nc.sync.dma_start(W_sb[:], W_in[:].rearrange("l p j -> p l j"))
            if flags["bias"]:
                bs_sb = const_pool.tile([128, c.L, D], F32)
                nc.sync.dma_start(bs_sb[:], bs_in[:].rearrange("l p j -> p l j"))
            if flags["affine"]:
                lng_sb = const_pool.tile([128, c.L - 1, D], F32)
                nc.sync.dma_start(lng_sb[:], lng_in[:].rearrange("l p j -> p l j"))
                lnb_sb = const_pool.tile([128, c.L - 1, D], F32)
                nc.sync.dma_start(lnb_sb[:], lnb_in[:].rearrange("l p j -> p l j"))

            # ---- persistent state ----
            x_state = state_pool.tile([128, NT, D], F32)
            agg = state_pool.tile([128, NT, D], F32)
            hp_bf = state_pool.tile([128, NT, 64], MSG)

            nc.sync.dma_start(x_state[:], x_in[:].rearrange("p (t f) -> p t f", f=D))

            for layer in range(c.L):
                # ======== 1) h' = x @ W  (own shard) ========
                for q in range(c.n_chunks):
                    st = q * qt
                    q_end = (q + 1) * qt
                    while st < q_end:
                        n_t = min(4, q_end - st)
                        xdT = xT_pool.tile([64, 4, 128], F32, tag="xdT")
                        for j in range(n_t):
                            t = st + j
                            ptr = ptr_pool.tile([64, 128], F32, tag="ptr")
                            nc.tensor.transpose(ptr[:], x_state[:, t, :], i128[:])
                            nc.scalar.copy(xdT[:, j, :], ptr[:])
                        hT_ps = pbig_pool.tile([64, 4 * 128], F32)
                        nc.tensor.matmul(
                            hT_ps[:, :n_t * 128],
                            W_sb[:, layer, :],
                            xdT[:, :n_t, :],
                            start=True, stop=True)
                        hT_sb = xT_pool.tile([64, 4, 128], F32, tag="hT")
                        nc.scalar.copy(hT_sb[:, :n_t, :], hT_ps[:, :n_t * 128].rearrange("p (a b) -> p a b", b=128))
                        for j in range(n_t):
                            t = st + j
                            ptr2 = ptr_pool.tile([128, 64], F32, tag="ptr")
                            nc.tensor.transpose(ptr2[:], hT_sb[:, j, :], i64[:])
                            nc.vector.tensor_copy(hp_bf[:, t, :], ptr2[:])
                        st += n_t
                    # store this quarter (flat 2D, partition-major) and kick its AG
                    nc.sync.dma_start(
                        hp_shards[layer][q][:, :qt * 64],
                        hp_bf[:, q * qt:(q + 1) * qt, :].rearrange("p t f -> p (t f)"))
                    # high_priority pins the trigger ahead of the gather
                    # stream in the GpSimd engine order - otherwise the tile
                    # scheduler parks it behind the previous chunk's gathers
                    # and the AllGathers serialize with them.
                    with tc.high_priority():
                        nc.gpsimd.collective_compute(
                            "AllGather", AOP.bypass,
                            replica_groups=[list(range(c.n_cores))],
                            ins=[hp_shards[layer][q][:].opt()],
                            outs=[hp_fulls[layer][q][:].opt()],
                        )

                # ======== 2) gather + one-hot matmul reduction ========
                gb = 0               # global block id
                s_tile = None
                ps = None
                seg_ctr = 0          # rotates gather calls across SWDGE queues
                if layer == 0:
                    nidx_regs = {}
                for ck in range(c.n_chunks):
                    ck_blocks = []   # (w, idx_in_group, group_size)
                    for w in range(NT):
                        for i in range(int(blocks[ck, w])):
                            ck_blocks.append((w, i, int(blocks[ck, w])))
                    nb_ck = len(ck_blocks)
                    if nb_ck == 0:
                        continue
                    in_rows = hp_fulls[layer][ck][:]
                    n_seg = (nb_ck + c.segblk - 1) // c.segblk
                    gb0_ck = gb
                    for s in range(n_seg):
                        b0 = s * c.segblk
                        nblk_s = min(c.segblk, nb_ck - b0)
                        nidx = nblk_s * 128
                        goff = (gb0_ck + b0) * 8       # idx cols (8 per 128-edge block)
                        msg = msg_pool.tile([128, c.segblk, ROWE], MSG)
                        if nidx not in nidx_regs:
                            nidx_regs[nidx] = nc.gpsimd.to_reg(nidx)
                        nc.gpsimd.dma_gather(
                            msg[:, :nblk_s, :], in_rows,
                            idx_all[:, goff:goff + nblk_s * 8],
                            nidx, nidx_regs[nidx], ROWE,
                            single_packet=(nidx <= 1024 or c.force_single_packet),
                            queue_num=seg_ctr % c.n_queues)
                        seg_ctr += 1
                        # segment's parity-split weighted S tiles, one 2D DMA
                        g0 = gb + b0
                        s_tile = s_pool.tile([128, c.segblk * 256], MSG, tag="S")
                        nc.scalar.dma_start(
                            s_tile[:, :nblk_s * 256],
                            S_in[:, g0 * 256:(g0 + nblk_s) * 256])
                        for bl in range(nblk_s):
                            w, gi, gsz = ck_blocks[b0 + bl]
                            if gi == 0:
                                ps = pagg_pool.tile([128, D], F32)
                            nc.tensor.matmul(
                                ps[:], s_tile[:, bl * 256:bl * 256 + 128],
                                msg[:, bl, 0:64],
                                start=(gi == 0), stop=False)
                            nc.tensor.matmul(
                                ps[:], s_tile[:, bl * 256 + 128:bl * 256 + 256],
                                msg[:, bl, 64:128],
                                start=False, stop=(gi == gsz - 1))
                            if gi == gsz - 1:
                                if first_ck[w] == ck:
                                    nc.scalar.copy(agg[:, w, :], ps[:])
                                else:
                                    nc.vector.tensor_tensor(
                                        agg[:, w, :], agg[:, w, :], ps[:], AOP.add)
                    gb += nb_ck

                # windows with no edges at all
                for w in range(NT):
                    if first_ck[w] is None:
                        nc.vector.memset(agg[:, w, :], 0.0)

                # ======== 3) epilogue ========
                # agg += dinv^2 * hp_own  (self-loop); x += agg
                for t in range(NT):
                    nc.vector.scalar_tensor_tensor(
                        agg[:, t, :], hp_bf[:, t, :], dinv2T[:, t:t + 1],
                        agg[:, t, :], AOP.mult, AOP.add)
                nc.vector.tensor_tensor(x_state[:], x_state[:], agg[:], AOP.add)
                if flags["bias"]:
                    for t in range(NT):
                        nc.vector.tensor_tensor(
                            x_state[:, t, :], x_state[:, t, :],
                            bs_sb[:, layer, :], AOP.add)

                if layer == c.L - 1:
                    # emb = x (pre-relu); x_out = relu(emb)
                    nc.sync.dma_start(
                        emb_out[:], x_state[:].rearrange("p t f -> p (t f)"))
                    nc.vector.tensor_scalar(
                        agg[:], x_state[:], 0.0, None, AOP.max)
                    nc.sync.dma_start(
                        x_out[:], agg[:].rearrange("p t f -> p (t f)"))
                else:
                    # relu in place
                    nc.vector.tensor_scalar(
                        x_state[:], x_state[:], 0.0, None, AOP.max)
                    # LayerNorm (batched stats)
                    mu = stats_pool.tile([128, NT], F32, tag="mu")
                    ss = stats_pool.tile([128, NT], F32, tag="ss")
                    rstd = stats_pool.tile([128, NT], F32, tag="rstd")
                    vtmp = stats_pool.tile([128, NT], F32, tag="vtmp")
                    nc.vector.tensor_reduce(mu[:], x_state[:], mybir.AxisListType.X, AOP.add)
                    nc.scalar.activation(agg[:], x_state[:], mybir.ActivationFunctionType.Square)
                    nc.vector.tensor_reduce(ss[:], agg[:], mybir.AxisListType.X, AOP.add)
                    nc.vector.tensor_scalar(mu[:], mu[:], 1.0 / D, None, AOP.mult)
                    nc.vector.tensor_tensor(vtmp[:], mu[:], mu[:], AOP.mult)
                    nc.vector.scalar_tensor_tensor(
                        vtmp[:], ss[:], 1.0 / D, vtmp[:], AOP.mult, AOP.subtract)
                    nc.scalar.activation(vtmp[:], vtmp[:], mybir.ActivationFunctionType.Sqrt,
                                         bias=eps_sb[:])
                    nc.vector.reciprocal(rstd[:], vtmp[:])
                    for t in range(NT):
                        nc.vector.tensor_scalar(
                            x_state[:, t, :], x_state[:, t, :],
                            mu[:, t:t + 1], rstd[:, t:t + 1],
                            AOP.subtract, AOP.mult)
                    if flags["affine"]:
                        for t in range(NT):
                            nc.vector.tensor_tensor(
                                x_state[:, t, :], x_state[:, t, :],
                                lng_sb[:, layer, :], AOP.mult)
                            nc.vector.tensor_tensor(
                                x_state[:, t, :], x_state[:, t, :],
                                lnb_sb[:, layer, :], AOP.add)

    nc.compile()
    return nc


_CACHE = {}
last_results = None


def _run(cfg, inputs, trace=False):
    global last_results
    per_core, consts, struct, flags = _prep(cfg, **inputs)
    key = (cfg.N, cfg.E, cfg.msg_bf16, struct["nblk_total"], cfg.n_queues)
    if key not in _CACHE:
        _CACHE[key] = _build(cfg, struct, flags)
    nc = _CACHE[key]
    in_maps = []
    for cc in range(cfg.n_cores):
        m = dict(consts)
        m.update(per_core[cc])
        in_maps.append(m)
    res = run_bass_kernel_spmd(nc, in_maps, list(range(cfg.n_cores)), trace=trace)
    last_results = res

    def unshard(name):
        parts = []
        for r in res.results:
            a = np.asarray(r[name]).reshape(128, cfg.nt, cfg.D)
            parts.append(a.transpose(1, 0, 2).reshape(cfg.t_pad, cfg.D)[:cfg.npc])
        return np.concatenate(parts, axis=0)

    return unshard("emb_out"), unshard("x_out")


def kernel(x, edge_index, Ws, bs, ln_g, ln_b):
    cfg = Cfg(msg_bf16=os.environ.get("GCN_MSG_F32", "0") != "1",
              seg_edges=int(os.environ.get("GCN_SEG", "3072")),
              dma_scratch=int(os.environ.get("GCN_SCRATCH", "16384")),
              force_single_packet=os.environ.get("GCN_SP", "0") == "1",
              n_queues=int(os.environ.get("GCN_NQ", "4")))
    return _run(cfg, dict(x=x, edge_index=edge_index, Ws=Ws, bs=bs,
                          ln_g=ln_g, ln_b=ln_b),
                trace=os.environ.get("GCN_TRACE", "0") == "1")



# revision 43
# speedup vs baseline: 1.0011x; 1.0011x over previous
"""Trainium2 Bass kernel for a 3-layer GCN (directional, symmetric-norm,
self-loops, skip connections, LayerNorm between layers).

Strategy (8 NeuronCores, SPMD, single NEFF launch):
  - Nodes are sharded by destination across the 8 cores (12500 each).
  - Per layer, each core computes its shard of h' = x @ W, writes it to HBM
    partition-major (so stores are flat 2D DMAs), and a per-quarter
    AllGather replicates the full h' table to every core.
  - h' rows are packed at 128B and gathered as 256B node-PAIRS, so a
    source chunk of 50176 nodes is 25088 gather rows and fits int16:
    only 2 chunks -> 2 AllGathers per layer (the serialized AG chain was
    a main bottleneck at 4), and ~12% less block padding.
  - Edges are bucketed on the host by (source-chunk, dest-window), with
    destinations in 128-node windows.
  - Messages are fetched with the GPSIMD dma_gather custom instruction;
    the gather calls round-robin over 4 SWDGE queues so descriptor
    generation runs on all four Q7 core pairs concurrently (queue q runs
    on Q7 cores {2q, 2q+1} - the single-queue default serializes at
    ~8 ns/edge and dominates the whole kernel).
  - Aggregation per 128-edge block is a pair of weighted one-hot matmuls
    on the TensorEngine (even/odd parity halves of the gathered pair):
    S[par][e, d] = norm(e) for d == dest_rel(e), zero elsewhere. S tiles
    are built ON THE HOST (carrying the dinv[row]*dinv[col] edge weights)
    and streamed from HBM via the Scalar-engine HWDGE, which removes all
    DVE is_equal/scale work from the inner loop. PSUM accumulates per
    (chunk, window) group into an SBUF accumulator.
  - The gather index table is layer-invariant and loaded to SBUF once.
  - Self-loops use the local bf16 h' copy scaled by dinv^2; the epilogue
    (skip add, ReLU, LayerNorm) runs node-major, stats batched per layer.

The per-(chunk,window) block counts are made identical across cores by
padding to the max (pad slots gather row 0 with zero S weight), so a single
program runs on all 8 cores with per-core data only.
"""

import math
import os
import sys

import numpy as np

for _p in ("/opt/trn_rl_repo",):
    if os.path.isdir(_p) and _p not in sys.path:
        sys.path.insert(0, _p)

import concourse.bacc as bacc
import concourse.bass as bass
import concourse.mybir as mybir
import concourse.tile as tile
from concourse.bass_utils import run_bass_kernel_spmd

try:
    from ml_dtypes import bfloat16 as np_bf16
except ImportError:  # pragma: no cover
    np_bf16 = mybir.dt.np(mybir.dt.bfloat16)

F32 = mybir.dt.float32
BF16 = mybir.dt.bfloat16
I16 = mybir.dt.int16
AOP = mybir.AluOpType


class Cfg:
    def __init__(self, N=100000, E=1600000, D=64, L=3, n_cores=8,
                 seg_edges=8192, sblk=8, msg_bf16=True, eps=1e-5,
                 dma_scratch=16384, force_single_packet=False, n_queues=4):
        self.dma_scratch = dma_scratch
        self.force_single_packet = force_single_packet
        self.n_queues = n_queues
        assert N % n_cores == 0
        self.N, self.E, self.D, self.L, self.n_cores = N, E, D, L, n_cores
        self.eps = eps
        self.npc = N // n_cores                     # nodes per core
        nt0 = (self.npc + 127) // 128               # 128-node tiles (= windows)
        nch = max(1, math.ceil(nt0 * 128 * n_cores / 65536))
        self.nt = ((nt0 + nch - 1) // nch) * nch    # quarter-aligned tiles
        self.t_pad = self.nt * 128                  # padded shard size
        self.n_chunks = nch
        self.qsz = self.t_pad // nch                # rows per source-quarter
        self.qsz_ag = self.qsz
        self.chunk = self.qsz_ag * n_cores          # rows per gather table
        assert self.chunk // 2 <= 32768 and self.qsz % 128 == 0
        self.seg_edges = seg_edges                  # edges per dma_gather call
        assert seg_edges % 128 == 0
        self.segblk = seg_edges // 128
        self.sblk = sblk                            # S-tiles built per DVE op
        self.msg_bf16 = msg_bf16
        self.msg_dt = BF16 if msg_bf16 else F32
        self.row_elems = 128 if msg_bf16 else 64    # gather elem_size (=256B)


def _prep(cfg, x, edge_index, Ws, bs, ln_g, ln_b):
    """Host-side preprocessing: degrees, edge bucketing, per-core arrays."""
    c = cfg
    row = np.asarray(edge_index[0], dtype=np.int64)
    col = np.asarray(edge_index[1], dtype=np.int64)
    deg = np.bincount(row, minlength=c.N).astype(np.float64) + 1.0
    dinv = (1.0 / np.sqrt(deg)).astype(np.float32)

    norm = (dinv[row] * dinv[col]).astype(np.float32)  # per-edge weight

    core = row // c.npc
    row_local = row - core * c.npc
    win = row_local >> 7
    row_rel = row_local & 127
    # source position in the partition-major gather table: node (tile t,
    # partition p) lands at row p*qt + (t % qt) of its quarter's shard
    qt_t = c.nt // c.n_chunks                         # tiles per quarter
    qt_ag = c.qsz_ag // 128                           # tiles incl AG pad
    src_core = col // c.npc
    src_loc = col % c.npc
    s_t = src_loc // 128
    s_p = src_loc % 128
    chunk = s_t // qt_t                               # source quarter
    pos = src_core * c.qsz_ag + s_p * qt_ag + (s_t % qt_t)
    idx_rel = pos >> 1                                # 256B row = node pair
    parity = (pos & 1).astype(np.int64)

    # bucket key and stable sort
    key = (core * c.n_chunks + chunk) * c.nt + win
    order = np.argsort(key, kind="stable")
    key_s = key[order]
    idx_s = idx_rel[order].astype(np.int32)
    rr_s = row_rel[order].astype(np.int32)
    nm_s = norm[order]
    par_s = parity[order]

    nbuck = c.n_cores * c.n_chunks * c.nt
    counts = np.bincount(key_s, minlength=nbuck).reshape(c.n_cores, c.n_chunks, c.nt)
    starts_flat = np.zeros(nbuck + 1, dtype=np.int64)
    np.cumsum(counts.reshape(-1), out=starts_flat[1:])

    # common (max over cores) block counts per (chunk, window)
    blocks = np.ceil(counts.max(axis=0) / 128.0).astype(np.int64)  # [n_chunks, nt]
    nblk_total = int(blocks.sum())
    e_pad = nblk_total * 128
    blk_off = np.zeros(c.n_chunks * c.nt + 1, dtype=np.int64)
    np.cumsum(blocks.reshape(-1), out=blk_off[1:])

    # per-core padded edge streams (pad edges: idx 0; S weight 0) and
    # host-built weighted one-hot S tiles: S[e%128, blk, dest] = norm(e)
    msg_np = np_bf16 if c.msg_bf16 else np.float32
    idx_arrs, s_arrs = [], []
    for cc in range(c.n_cores):
        idx_a = np.zeros(e_pad, dtype=np.int16)
        rr_a = np.zeros(e_pad, dtype=np.int64)
        nm_a = np.zeros(e_pad, dtype=np.float32)
        pa_a = np.zeros(e_pad, dtype=np.int64)
        for ck in range(c.n_chunks):
            for w in range(c.nt):
                b = starts_flat[(cc * c.n_chunks + ck) * c.nt + w]
                e = starts_flat[(cc * c.n_chunks + ck) * c.nt + w + 1]
                n = e - b
                o = blk_off[ck * c.nt + w] * 128
                idx_a[o:o + n] = idx_s[b:e].astype(np.int16)
                rr_a[o:o + n] = rr_s[b:e]
                nm_a[o:o + n] = nm_s[b:e]
                pa_a[o:o + n] = par_s[b:e]
        idx_arrs.append(idx_a)
        # parity-split weighted one-hot: S[par][e%128, blk, dest] = norm(e)
        S = np.zeros((128, nblk_total, 2, 128), dtype=msg_np)
        slot = np.arange(e_pad, dtype=np.int64)
        S[slot % 128, slot // 128, pa_a, rr_a] = nm_a.astype(msg_np)
        s_arrs.append(S)

    # wrap indices for dma_gather: flat i -> [i%16, i//16], replicated to 128 partitions
    def wrap_idx(a):
        return np.tile(a.reshape(-1, 16).T, (8, 1)).astype(np.int16)

    per_core = []
    x = np.asarray(x, dtype=np.float32)
    for cc in range(c.n_cores):
        xs = np.zeros((c.t_pad, c.D), dtype=np.float32)
        xs[:c.npc] = x[cc * c.npc:(cc + 1) * c.npc]
        # partition-major x: [128, nt*64], row p holds tiles' node t*128+p
        xpm = np.ascontiguousarray(
            xs.reshape(c.nt, 128, c.D).transpose(1, 0, 2)).reshape(128, c.nt * c.D)
        dl = np.zeros(c.t_pad, dtype=np.float32)
        dl[:c.npc] = dinv[cc * c.npc:(cc + 1) * c.npc]
        d2 = (dl * dl).astype(np.float32)
        per_core.append({
            "x_in": xpm,
            "dinv2_in": d2.reshape(c.nt, 128).T.copy(),     # [128, nt]
            "idxs_in": wrap_idx(idx_arrs[cc]),              # [128, e_pad//16]
            "S_in": s_arrs[cc].reshape(128, nblk_total * 256),  # [128, nblk*2*128]
        })

    consts = {
        "W_in": np.ascontiguousarray(np.asarray(Ws, dtype=np.float32)),   # [L,64,64]
        "i64_in": np.eye(64, dtype=np.float32),
        "i128_in": np.eye(128, dtype=np.float32),
    }
    bs = np.asarray(bs, dtype=np.float32)
    ln_g = np.asarray(ln_g, dtype=np.float32)
    ln_b = np.asarray(ln_b, dtype=np.float32)
    flags = {
        "bias": bool(np.any(bs != 0.0)),
        "affine": bool(np.any(ln_g != 1.0) or np.any(ln_b != 0.0)),
    }
    if flags["bias"]:
        consts["bs_in"] = np.tile(bs[:, None, :], (1, 128, 1))        # [L,128,64]
    if flags["affine"]:
        consts["lng_in"] = np.tile(ln_g[:, None, :], (1, 128, 1))     # [L-1,128,64]
        consts["lnb_in"] = np.tile(ln_b[:, None, :], (1, 128, 1))
    struct = {
        "blocks": blocks,            # [n_chunks, nt]
        "nblk_total": nblk_total,
        "e_pad": e_pad,
    }
    return per_core, consts, struct, flags


def _build(cfg, struct, flags):
    """Build the Bass/Tile program. Returns nc."""
    c = cfg
    blocks = struct["blocks"]
    nblk_total = struct["nblk_total"]
    e_pad = struct["e_pad"]
    D = c.D
    NT = c.nt
    MSG = c.msg_dt
    ROWE = c.row_elems

    # first chunk contributing each window (for copy-vs-add into agg)
    first_ck = [None] * NT
    for w in range(NT):
        for ck in range(c.n_chunks):
            if blocks[ck, w] > 0:
                first_ck[w] = ck
                break

    nc = bacc.Bacc("TRN2", num_devices=c.n_cores, target_bir_lowering=False,
                   debug=False, enable_asserts=False,
                   dynamic_dma_scratch_size=c.dma_scratch,
                   num_swdge_queues=c.n_queues)

    # I/O
    x_in = nc.dram_tensor("x_in", [128, (c.t_pad // 128) * D], F32, kind="ExternalInput")
    dinv2_in = nc.dram_tensor("dinv2_in", [128, NT], F32, kind="ExternalInput")
    idxs_in = nc.dram_tensor("idxs_in", [128, e_pad // 16], I16, kind="ExternalInput")
    S_in = nc.dram_tensor("S_in", [128, nblk_total * 256], MSG, kind="ExternalInput")
    W_in = nc.dram_tensor("W_in", [c.L, D, D], F32, kind="ExternalInput")
    i64_in = nc.dram_tensor("i64_in", [64, 64], F32, kind="ExternalInput")
    i128_in = nc.dram_tensor("i128_in", [128, 128], F32, kind="ExternalInput")
    if flags["bias"]:
        bs_in = nc.dram_tensor("bs_in", [c.L, 128, D], F32, kind="ExternalInput")
    if flags["affine"]:
        lng_in = nc.dram_tensor("lng_in", [c.L - 1, 128, D], F32, kind="ExternalInput")
        lnb_in = nc.dram_tensor("lnb_in", [c.L - 1, 128, D], F32, kind="ExternalInput")
    emb_out = nc.dram_tensor("emb_out", [128, (c.t_pad // 128) * D], F32, kind="ExternalOutput")
    x_out = nc.dram_tensor("x_out", [128, (c.t_pad // 128) * D], F32, kind="ExternalOutput")

    with tile.TileContext(nc) as tc:
        with (
            tc.tile_pool(name="dram", bufs=1, space="DRAM") as dram_pool,
            tc.tile_pool(name="const", bufs=1) as const_pool,
            tc.tile_pool(name="state", bufs=1) as state_pool,
            tc.tile_pool(name="xd", bufs=4) as xd_pool,
            tc.tile_pool(name="xT", bufs=2) as xT_pool,
            tc.tile_pool(name="msg", bufs=7) as msg_pool,
            tc.tile_pool(name="sS", bufs=4) as s_pool,
            tc.tile_pool(name="stats", bufs=2) as stats_pool,
            tc.tile_pool(name="pagg", bufs=5, space="PSUM") as pagg_pool,
            tc.tile_pool(name="pbig", bufs=1, space="PSUM") as pbig_pool,
            tc.tile_pool(name="ptr", bufs=2, space="PSUM") as ptr_pool,
        ):
            # ---- DRAM internal buffers for the halo exchange / gather ----
            # per-(layer, quarter) shard, partition-major: linear order is
            # (p, tile-in-quarter, col) so the AllGather rank-concat yields a
            # node-row-major gather table under the renumbered node ids.
            qt = NT // c.n_chunks
            qt_ag = c.qsz_ag // 128    # AG tiles per quarter (incl pad tiles)
            hp_shards = [
                [dram_pool.tile([128, qt_ag * 64], MSG, name=f"hp_shard{i}q{q}")
                 for q in range(c.n_chunks)]
                for i in range(c.L)]
            hp_fulls = [
                [dram_pool.tile([c.chunk // 2, ROWE], MSG, addr_space="Shared",
                                name=f"hp_full{i}q{q}")
                 for q in range(c.n_chunks)]
                for i in range(c.L)]

            # ---- constants ----
            dinv2T = const_pool.tile([128, NT], F32)
            nc.sync.dma_start(dinv2T[:], dinv2_in[:])
            idx_all = const_pool.tile([128, e_pad // 16], I16)
            nc.sync.dma_start(idx_all[:], idxs_in[:])
            i64 = const_pool.tile([64, 64], F32)
            nc.sync.dma_start(i64[:], i64_in[:])
            i128 = const_pool.tile([128, 128], F32)
            nc.sync.dma_start(i128[:], i128_in[:])
            eps_sb = const_pool.tile([128, 1], F32)
            nc.vector.memset(eps_sb[:], float(c.eps))
            W_sb = const_pool.tile([64, c.L, D], F32)
            nc.sync.dma_start(W_sb[:], W_in[:].rearrange("l p j -> p l j"))
            if flags["bias"]:
                bs_sb = const_pool.tile([128, c.L, D], F32)
                nc.sync.dma_start(bs_sb[:], bs_in[:].rearrange("l p j -> p l j"))
            if flags["affine"]:
                lng_sb = const_pool.tile([128, c.L - 1, D], F32)
                nc.sync.dma_start(lng_sb[:], lng_in[:].rearrange("l p j -> p l j"))
                lnb_sb = const_pool.tile([128, c.L - 1, D], F32)
                nc.sync.dma_start(lnb_sb[:], lnb_in[:].rearrange("l p j -> p l j"))

            # ---- persistent state ----
            x_state = state_pool.tile([128, NT, D], F32)
            agg = state_pool.tile([128, NT, D], F32)
            hp_bf = state_pool.tile([128, NT, 64], MSG)

            nc.sync.dma_start(x_state[:], x_in[:].rearrange("p (t f) -> p t f", f=D))

            for layer in range(c.L):
                # ======== 1) h' = x @ W  (own shard) ========
                for q in range(c.n_chunks):
                    st = q * qt
                    q_end = (q + 1) * qt
                    while st < q_end:
                        n_t = min(4, q_end - st)
                        xdT = xT_pool.tile([64, 4, 128], F32, tag="xdT")
                        for j in range(n_t):
                            t = st + j
                            ptr = ptr_pool.tile([64, 128], F32, tag="ptr")
                            nc.tensor.transpose(ptr[:], x_state[:, t, :], i128[:])
                            nc.scalar.copy(xdT[:, j, :], ptr[:])
                        hT_ps = pbig_pool.tile([64, 4 * 128], F32)
                        nc.tensor.matmul(
                            hT_ps[:, :n_t * 128],
                            W_sb[:, layer, :],
                            xdT[:, :n_t, :],
                            start=True, stop=True)
                        hT_sb = xT_pool.tile([64, 4, 128], F32, tag="hT")
                        nc.scalar.copy(hT_sb[:, :n_t, :], hT_ps[:, :n_t * 128].rearrange("p (a b) -> p a b", b=128))
                        for j in range(n_t):
                            t = st + j
                            ptr2 = ptr_pool.tile([128, 64], F32, tag="ptr")
                            nc.tensor.transpose(ptr2[:], hT_sb[:, j, :], i64[:])
                            nc.vector.tensor_copy(hp_bf[:, t, :], ptr2[:])
                        st += n_t
                    # store this quarter (flat 2D, partition-major) and kick its AG
                    nc.sync.dma_start(
                        hp_shards[layer][q][:, :qt * 64],
                        hp_bf[:, q * qt:(q + 1) * qt, :].rearrange("p t f -> p (t f)"))
                    nc.gpsimd.collective_compute(
                        "AllGather", AOP.bypass,
                        replica_groups=[list(range(c.n_cores))],
                        ins=[hp_shards[layer][q][:].opt()],
                        outs=[hp_fulls[layer][q][:].opt()],
                    )

                # ======== 2) gather + one-hot matmul reduction ========
                gb = 0               # global block id
                s_tile = None
                ps = None
                seg_ctr = 0          # rotates gather calls across SWDGE queues
                if layer == 0:
                    nidx_regs = {}
                for ck in range(c.n_chunks):
                    ck_blocks = []   # (w, idx_in_group, group_size)
                    for w in range(NT):
                        for i in range(int(blocks[ck, w])):
                            ck_blocks.append((w, i, int(blocks[ck, w])))
                    nb_ck = len(ck_blocks)
                    if nb_ck == 0:
                        continue
                    in_rows = hp_fulls[layer][ck][:]
                    n_seg = (nb_ck + c.segblk - 1) // c.segblk
                    gb0_ck = gb
                    for s in range(n_seg):
                        b0 = s * c.segblk
                        nblk_s = min(c.segblk, nb_ck - b0)
                        nidx = nblk_s * 128
                        goff = (gb0_ck + b0) * 8       # idx cols (8 per 128-edge block)
                        msg = msg_pool.tile([128, c.segblk, ROWE], MSG)
                        if nidx not in nidx_regs:
                            nidx_regs[nidx] = nc.gpsimd.to_reg(nidx)
                        nc.gpsimd.dma_gather(
                            msg[:, :nblk_s, :], in_rows,
                            idx_all[:, goff:goff + nblk_s * 8],
                            nidx, nidx_regs[nidx], ROWE,
                            single_packet=(nidx <= 1024 or c.force_single_packet),
                            queue_num=seg_ctr % c.n_queues)
                        seg_ctr += 1
                        # segment's parity-split weighted S tiles, one 2D DMA
                        g0 = gb + b0
                        s_tile = s_pool.tile([128, c.segblk * 256], MSG, tag="S")
                        nc.scalar.dma_start(
                            s_tile[:, :nblk_s * 256],
                            S_in[:, g0 * 256:(g0 + nblk_s) * 256])
                        for bl in range(nblk_s):
                            w, gi, gsz = ck_blocks[b0 + bl]
                            if gi == 0:
                                ps = pagg_pool.tile([128, D], F32)
                            nc.tensor.matmul(
                                ps[:], s_tile[:, bl * 256:bl * 256 + 128],
                                msg[:, bl, 0:64],
                                start=(gi == 0), stop=False)
                            nc.tensor.matmul(
                                ps[:], s_tile[:, bl * 256 + 128:bl * 256 + 256],
                                msg[:, bl, 64:128],
                                start=False, stop=(gi == gsz - 1))
                            if gi == gsz - 1:
                                if first_ck[w] == ck:
                                    nc.scalar.copy(agg[:, w, :], ps[:])
                                else:
                                    nc.vector.tensor_tensor(
                                        agg[:, w, :], agg[:, w, :], ps[:], AOP.add)
                    gb += nb_ck

                # windows with no edges at all
                for w in range(NT):
                    if first_ck[w] is None:
                        nc.vector.memset(agg[:, w, :], 0.0)

                # ======== 3) epilogue ========
                # agg += dinv^2 * hp_own  (self-loop); x += agg
                for t in range(NT):
                    nc.vector.scalar_tensor_tensor(
                        agg[:, t, :], hp_bf[:, t, :], dinv2T[:, t:t + 1],
                        agg[:, t, :], AOP.mult, AOP.add)
                nc.vector.tensor_tensor(x_state[:], x_state[:], agg[:], AOP.add)
                if flags["bias"]:
                    for t in range(NT):
                        nc.vector.tensor_tensor(
                            x_state[:, t, :], x_state[:, t, :],
                            bs_sb[:, layer, :], AOP.add)

                if layer == c.L - 1:
                    # emb = x (pre-relu); x_out = relu(emb)
                    nc.sync.dma_start(
                        emb_out[:], x_state[:].rearrange("p t f -> p (t f)"))
                    nc.vector.tensor_scalar(
                        agg[:], x_state[:], 0.0, None, AOP.max)
                    nc.sync.dma_start(
                        x_out[:], agg[:].rearrange("p t f -> p (t f)"))
                else:
                    # relu in place
                    nc.vector.tensor_scalar(
                        x_state[:], x_state[:], 0.0, None, AOP.max)
                    # LayerNorm (batched stats)
                    mu = stats_pool.tile([128, NT], F32, tag="mu")
                    ss = stats_pool.tile([128, NT], F32, tag="ss")
                    rstd = stats_pool.tile([128, NT], F32, tag="rstd")
                    vtmp = stats_pool.tile([128, NT], F32, tag="vtmp")
                    nc.vector.tensor_reduce(mu[:], x_state[:], mybir.AxisListType.X, AOP.add)
                    nc.scalar.activation(agg[:], x_state[:], mybir.ActivationFunctionType.Square)
                    nc.vector.tensor_reduce(ss[:], agg[:], mybir.AxisListType.X, AOP.add)
                    nc.vector.tensor_scalar(mu[:], mu[:], 1.0 / D, None, AOP.mult)
                    nc.vector.tensor_tensor(vtmp[:], mu[:], mu[:], AOP.mult)
                    nc.vector.scalar_tensor_tensor(
                        vtmp[:], ss[:], 1.0 / D, vtmp[:], AOP.mult, AOP.subtract)
                    nc.scalar.activation(vtmp[:], vtmp[:], mybir.ActivationFunctionType.Sqrt,
                                         bias=eps_sb[:])
                    nc.vector.reciprocal(rstd[:], vtmp[:])
                    for t in range(NT):
                        nc.vector.tensor_scalar(
                            x_state[:, t, :], x_state[:, t, :],
                            mu[:, t:t + 1], rstd[:, t:t + 1],
                            AOP.subtract, AOP.mult)
                    if flags["affine"]:
                        for t in range(NT):
                            nc.vector.tensor_tensor(
                                x_state[:, t, :], x_state[:, t, :],
                                lng_sb[:, layer, :], AOP.mult)
                            nc.vector.tensor_tensor(
                                x_state[:, t, :], x_state[:, t, :],
                                lnb_sb[:, layer, :], AOP.add)

    nc.compile()
    return nc


_CACHE = {}
last_results = None


def _run(cfg, inputs, trace=False):
    global last_results
    per_core, consts, struct, flags = _prep(cfg, **inputs)
    key = (cfg.N, cfg.E, cfg.msg_bf16, struct["nblk_total"], cfg.n_queues)
    if key not in _CACHE:
        _CACHE[key] = _build(cfg, struct, flags)
    nc = _CACHE[key]
    in_maps = []
    for cc in range(cfg.n_cores):
        m = dict(consts)
        m.update(per_core[cc])
        in_maps.append(m)
    res = run_bass_kernel_spmd(nc, in_maps, list(range(cfg.n_cores)), trace=trace)
    last_results = res

    def unshard(name):
        parts = []
        for r in res.results:
            a = np.asarray(r[name]).reshape(128, cfg.nt, cfg.D)
            parts.append(a.transpose(1, 0, 2).reshape(cfg.t_pad, cfg.D)[:cfg.npc])
        return np.concatenate(parts, axis=0)

    return unshard("emb_out"), unshard("x_out")


def kernel(x, edge_index, Ws, bs, ln_g, ln_b):
    cfg = Cfg(msg_bf16=os.environ.get("GCN_MSG_F32", "0") != "1",
              seg_edges=int(os.environ.get("GCN_SEG", "3072")),
              dma_scratch=int(os.environ.get("GCN_SCRATCH", "16384")),
              force_single_packet=os.environ.get("GCN_SP", "0") == "1",
              n_queues=int(os.environ.get("GCN_NQ", "4")))
    return _run(cfg, dict(x=x, edge_index=edge_index, Ws=Ws, bs=bs,
                          ln_g=ln_g, ln_b=ln_b),
                trace=os.environ.get("GCN_TRACE", "0") == "1")

